# revision 8
# baseline (speedup 1.0000x reference)
"""Trainium2 Bass kernel for DeepSeek-style MoE gate routing.

hidden_states [8, 4096, 2048] f32, w [256, 2048] f32, bias [256] f32
 -> topk_idx [32768, 8] int32, topk_weight [32768, 8] f32

Sharding: tokens split 8 ways across NeuronCores (4096 tokens/core); the
small gate weight + bias are replicated.  x is pre-transposed on the host so
the hidden dim lands on SBUF partitions with fully-contiguous DMA.

Matmul modes:
  fp32        - native fp32 matmuls (4 cyc/row).
  split3_bf16 - x and w split host-side into bf16 hi + bf16 lo;
                logits = xh*wh + xh*wl + xl*wh accumulated in one PSUM
                bank.  ~fp32-grade routing at bf16 matmul rate.

Self-contained: hardcodes all shapes; only imports the concourse toolchain.
"""
import sys

if "/opt/trn_rl_repo" not in sys.path:
    sys.path.insert(0, "/opt/trn_rl_repo")

import numpy as np

import concourse.bass as bass  # noqa: F401
import concourse.mybir as mybir
import concourse.tile as tile
from concourse import bacc
from concourse.bass_utils import run_bass_kernel_spmd

P = 128            # partitions / tokens per tile
H = 2048           # hidden dim
E = 256            # experts
KO = H // P        # 16 contraction chunks
N_CORES = 8
T_CORE = 4096      # tokens per core
N_TILES = T_CORE // P       # 32 token tiles per core
ST_TOK = 512                # tokens per super-tile
N_ST = T_CORE // ST_TOK     # 8 super-tiles
TPS = ST_TOK // P           # 4 tiles per super-tile

N_GROUP = 8
GSIZE = E // N_GROUP        # 32
TOPK_GROUP = 4
TOP_K = 8
SCALING = 2.5
NEG_BIG = -1.0e30

MATMUL_MODE = "split3_bf16"

f32 = mybir.dt.float32
f16 = mybir.dt.float16
bf16 = mybir.dt.bfloat16
u32 = mybir.dt.uint32
ALU = mybir.AluOpType
ACTF = mybir.ActivationFunctionType
AX = mybir.AxisListType

_CACHED_NC = {}


def build_kernel(mode=MATMUL_MODE):
    nc = bacc.Bacc("TRN2", target_bir_lowering=False, debug=False)

    if mode == "fp32":
        d_x = [nc.dram_tensor("xT", [H, T_CORE], f32, kind="ExternalInput")]
        d_w = [nc.dram_tensor("wT", [H, E], f32, kind="ExternalInput")]
        xdt = f32
    elif mode == "split3_bf16":
        d_x = [nc.dram_tensor("xTh", [H, T_CORE], bf16, kind="ExternalInput"),
               nc.dram_tensor("xTl", [H, T_CORE], bf16, kind="ExternalInput")]
        d_w = [nc.dram_tensor("wTh", [H, E], bf16, kind="ExternalInput"),
               nc.dram_tensor("wTl", [H, E], bf16, kind="ExternalInput")]
        xdt = bf16
    else:
        raise ValueError(mode)
    d_bias = nc.dram_tensor("biasrep", [P, E], f32, kind="ExternalInput")
    d_oidx = nc.dram_tensor("oidx", [P, N_TILES, TOP_K], u32, kind="ExternalOutput")
    d_owgt = nc.dram_tensor("owgt", [P, N_TILES, TOP_K], f32, kind="ExternalOutput")

    with tile.TileContext(nc) as tc:
        with tc.tile_pool(name="const", bufs=1) as cpool, \
             tc.tile_pool(name="xin", bufs=3) as xpool, \
             tc.tile_pool(name="score", bufs=2) as spool, \
             tc.tile_pool(name="small", bufs=2) as mpool, \
             tc.tile_pool(name="psum", bufs=4, space="PSUM") as ppool:

            # ---- constants ----
            if mode == "fp32":
                w_sb = cpool.tile([P, KO, E], f32, name="w0")
                nc.sync.dma_start(w_sb, d_w[0].ap().rearrange("(ko p) e -> p ko e", p=P))
            else:
                whl = cpool.tile([P, KO, 2 * E], bf16, name="whl")
                for kc in range(0, KO, 4):
                    nc.sync.dma_start(
                        whl[:, kc:kc + 4, :E],
                        d_w[0].ap().rearrange("(ko p) e -> p ko e", p=P)[:, kc:kc + 4])
                    nc.sync.dma_start(
                        whl[:, kc:kc + 4, E:],
                        d_w[1].ap().rearrange("(ko p) e -> p ko e", p=P)[:, kc:kc + 4])
            bias_sb = cpool.tile([P, E], f32)
            nc.sync.dma_start(bias_sb, d_bias.ap())
            negbig = cpool.tile([P, 1], f32)
            nc.vector.memset(negbig, NEG_BIG)
            mask_hi = cpool.tile([P, 1], u32)
            nc.vector.memset(mask_hi, 0xFFFFFF00)
            mask_lo = cpool.tile([P, 1], u32)
            nc.vector.memset(mask_lo, 0xFF)
            iota_e = cpool.tile([P, E], u32)
            nc.gpsimd.iota(iota_e, pattern=[[1, E]], base=0, channel_multiplier=0)
            oidx_sb = cpool.tile([P, N_TILES, TOP_K], u32)
            owgt_sb = cpool.tile([P, N_TILES, TOP_K], f32)

            # variable-size super-tiles: small first (fast start), small last (short tail)
            groups = [1, 3, 4, 4, 4, 4, 4, 4, 2, 1, 1]
            assert sum(groups) == N_TILES
            tl0 = 0
            for nt in groups:
                stok = nt * P
                x_sb = []
                for i, d in enumerate(d_x):
                    t = xpool.tile([P, KO, stok], xdt, tag=f"x{i}")
                    src_ap = d.ap().rearrange("(ko p) t -> p ko t", p=P)
                    nh = 2 if nt >= 2 else 1
                    half = stok // nh
                    for hh in range(nh):
                        nc.sync.dma_start(
                            t[:, :, hh * half:(hh + 1) * half],
                            src_ap[:, :, tl0 * P + hh * half:
                                   tl0 * P + (hh + 1) * half])
                    x_sb.append(t)

                # super-tile score tensors [128, nt, 256]
                sg_st = spool.tile([P, nt, E], f32, tag="sg")
                sb_st = spool.tile([P, nt, E], f32, tag="sb")
                sq_st = spool.tile([P, nt, E], f32, tag="sq")
                msf_st = spool.tile([P, nt, E], f32, tag="msf")
                zap_st = spool.tile([P, nt, E], f32, tag="zap")
                ssel_st = spool.tile([P, nt, E], f32, tag="ssel")
                t1g = mpool.tile([P, nt, N_GROUP], f32, tag="t1g")
                t2g = mpool.tile([P, nt, N_GROUP], f32, tag="t2g")
                gs = mpool.tile([P, nt, N_GROUP], f32, tag="gs")
                cc = mpool.tile([P, nt, N_GROUP, N_GROUP], f32, tag="cc")
                c8 = mpool.tile([P, nt, N_GROUP], f32, tag="c8")
                madd = mpool.tile([P, nt, N_GROUP], f32, tag="madd")
                v8 = mpool.tile([P, nt, 8], f32, tag="v8")
                s8 = mpool.tile([P, nt, 8], f32, tag="s8")
                is8 = mpool.tile([P, nt, 8], u32, tag="is8")
                eq = mpool.tile([P, nt, 8, 8], f32, tag="eq")
                sr3 = mpool.tile([P, nt, 8, 8], f32, tag="sr3")
                srank = mpool.tile([P, nt, 8], f32, tag="srank")
                ssum = mpool.tile([P, nt, 1], f32, tag="ssum")
                rs = mpool.tile([P, nt, 1], f32, tag="rs")

                for j in range(nt):
                    tl = tl0 + j
                    tsl = slice(j * P, (j + 1) * P)

                    # ---- logits ----
                    if mode == "fp32":
                        ps = ppool.tile([P, E], f32, tag="ps")
                        for k in range(KO):
                            nc.tensor.matmul(
                                ps, lhsT=x_sb[0][:, k, tsl], rhs=w_sb[:, k, :],
                                start=(k == 0), stop=(k == KO - 1))
                        sig_src = ps
                    else:
                        ps = ppool.tile([P, E], f32, tag="ps")
                        xh, xl = x_sb
                        for k in range(KO):
                            # all three split products accumulate into one bank
                            nc.tensor.matmul(
                                ps, lhsT=xh[:, k, tsl], rhs=whl[:, k, :E],
                                start=(k == 0), stop=False)
                            nc.tensor.matmul(
                                ps, lhsT=xh[:, k, tsl], rhs=whl[:, k, E:],
                                start=False, stop=False)
                            nc.tensor.matmul(
                                ps, lhsT=xl[:, k, tsl], rhs=whl[:, k, :E],
                                start=False, stop=(k == KO - 1))
                        sig_src = ps

                    # ---- sigma = sigmoid(logits) on ACT ----
                    nc.scalar.activation(sg_st[:, j, :], sig_src, ACTF.Sigmoid)

                    # scores_for_choice = sigma + bias            (GPSIMD)
                    nc.gpsimd.tensor_add(sb_st[:, j, :], sg_st[:, j, :], bias_sb)

                # sigma_q: low 8 mantissa bits <- expert id (batched DVE)
                nc.vector.scalar_tensor_tensor(
                    sq_st.bitcast(u32), sg_st.bitcast(u32),
                    mask_hi, iota_e[:, None, :].to_broadcast([P, nt, E]),
                    op0=ALU.bitwise_and, op1=ALU.bitwise_or)

                # ---- group top-2 (batched reduce + per-tile match_replace) ----
                sb4 = sb_st.rearrange("p t (g e) -> p t g e", g=N_GROUP)
                nc.vector.tensor_reduce(out=t1g, in_=sb4, axis=AX.X, op=ALU.max)
                for j in range(nt):
                    nc.vector.match_replace(
                        out=zap_st[:, j, :], in_to_replace=t1g[:, j, :],
                        in_values=sb_st[:, j, :], imm_value=NEG_BIG)
                nc.vector.tensor_reduce(
                    out=t2g, in_=zap_st.rearrange("p t (g e) -> p t g e", g=N_GROUP),
                    axis=AX.X, op=ALU.max)
                nc.vector.tensor_add(gs, t1g, t2g)

                # ---- group rank count + additive mask ----
                nc.vector.tensor_tensor(
                    out=cc,
                    in0=gs[:, :, None, :].to_broadcast([P, nt, N_GROUP, N_GROUP]),
                    in1=gs[:, :, :, None].to_broadcast([P, nt, N_GROUP, N_GROUP]),
                    op=ALU.is_gt)
                nc.vector.tensor_reduce(out=c8, in_=cc, axis=AX.X, op=ALU.add)
                nc.vector.scalar_tensor_tensor(
                    madd, c8, float(TOPK_GROUP) - 0.5,
                    negbig[:, :, None].to_broadcast([P, nt, N_GROUP]),
                    op0=ALU.is_gt, op1=ALU.mult)

                # ---- masked scores ----
                nc.vector.tensor_add(
                    msf_st.rearrange("p t (g e) -> p t g e", g=N_GROUP),
                    sb4,
                    madd[:, :, :, None].to_broadcast([P, nt, N_GROUP, GSIZE]))

                for j in range(nt):
                    tl = tl0 + j
                    # ---- top-8 of masked scores ----
                    nc.vector.max(out=v8[:, j, :], in_=msf_st[:, j, :])
                    nc.vector.max_index(out=oidx_sb[:, tl, :], in_max=v8[:, j, :],
                                        in_values=msf_st[:, j, :])
                    # ---- selected sigma_q: (msf >= v8[7]) * sigma_q ----
                    nc.vector.scalar_tensor_tensor(
                        ssel_st[:, j, :], msf_st[:, j, :], v8[:, j, 7:8],
                        sq_st[:, j, :], op0=ALU.is_ge, op1=ALU.mult)
                    nc.vector.max(out=s8[:, j, :], in_=ssel_st[:, j, :])

                # ---- decode embedded ids, reorder sigmas to score-rank order ----
                nc.vector.tensor_scalar(
                    out=is8, in0=s8.bitcast(u32), scalar1=mask_lo, scalar2=None,
                    op0=ALU.bitwise_and)
                nc.vector.tensor_tensor(
                    out=eq,
                    in0=oidx_sb[:, tl0:tl0 + nt, :, None]
                        .to_broadcast([P, nt, 8, 8]),
                    in1=is8[:, :, None, :].to_broadcast([P, nt, 8, 8]),
                    op=ALU.is_equal)
                nc.vector.tensor_tensor(
                    out=sr3, in0=eq,
                    in1=s8[:, :, None, :].to_broadcast([P, nt, 8, 8]),
                    op=ALU.mult)
                nc.vector.tensor_reduce(out=srank, in_=sr3, axis=AX.X, op=ALU.add)

                # ---- normalize * 2.5 ----
                nc.vector.tensor_reduce(out=ssum, in_=srank, axis=AX.X, op=ALU.add)
                nc.vector.reciprocal(rs, ssum)
                nc.vector.scalar_tensor_tensor(
                    owgt_sb[:, tl0:tl0 + nt, :], srank, SCALING,
                    rs.to_broadcast([P, nt, 8]),
                    op0=ALU.mult, op1=ALU.mult)

                ssl = slice(tl0, tl0 + nt)
                nc.sync.dma_start(d_oidx.ap()[:, ssl, :], oidx_sb[:, ssl, :])
                nc.sync.dma_start(d_owgt.ap()[:, ssl, :], owgt_sb[:, ssl, :])
                tl0 += nt

    nc.compile()
    return nc


def _get_nc(mode):
    if mode not in _CACHED_NC:
        _CACHED_NC[mode] = build_kernel(mode)
    return _CACHED_NC[mode]


def kernel(hidden_states, w, e_score_correction_bias, mode=MATMUL_MODE):
    T = hidden_states.shape[0] * hidden_states.shape[1]
    assert T == N_CORES * T_CORE
    x2 = np.ascontiguousarray(hidden_states.reshape(T, H).astype(np.float32))
    xT = np.ascontiguousarray(x2.T)                       # [H, T]
    wT = np.ascontiguousarray(np.asarray(w, np.float32).T)  # [H, E]
    bias_rep = np.ascontiguousarray(
        np.repeat(np.asarray(e_score_correction_bias, np.float32)[None, :], P, 0))

    if mode == "fp32":
        xs = {"xT": xT}
        ws = {"wT": wT}
    else:
        import ml_dtypes
        bf = ml_dtypes.bfloat16
        xh = xT.astype(bf)
        xl = (xT - xh.astype(np.float32)).astype(bf)
        whh = wT.astype(bf)
        wll = (wT - whh.astype(np.float32)).astype(bf)
        xs = {"xTh": xh, "xTl": xl}
        ws = {"wTh": whh, "wTl": wll}

    nc = _get_nc(mode)
    in_maps = []
    for c in range(N_CORES):
        m = {k: np.ascontiguousarray(v[:, c * T_CORE:(c + 1) * T_CORE])
             for k, v in xs.items()}
        m.update(ws)
        m["biasrep"] = bias_rep
        in_maps.append(m)

    res = run_bass_kernel_spmd(nc, in_maps, core_ids=list(range(N_CORES)))

    idx_parts, wgt_parts = [], []
    for c in range(N_CORES):
        r = res.results[c]
        idx_parts.append(r["oidx"].transpose(1, 0, 2).reshape(T_CORE, TOP_K))
        wgt_parts.append(r["owgt"].transpose(1, 0, 2).reshape(T_CORE, TOP_K))
    topk_idx = np.concatenate(idx_parts, 0).astype(np.int32)
    topk_weight = np.concatenate(wgt_parts, 0).astype(np.float32)
    return topk_idx, topk_weight


# revision 9
# speedup vs baseline: 1.0376x; 1.0376x over previous
"""Trainium2 Bass kernel for DeepSeek-style MoE gate routing.

hidden_states [8, 4096, 2048] f32, w [256, 2048] f32, bias [256] f32
 -> topk_idx [32768, 8] int32, topk_weight [32768, 8] f32

Sharding: tokens split 8 ways across NeuronCores (4096 tokens/core); the
small gate weight + bias are replicated.  x is pre-transposed on the host so
the hidden dim lands on SBUF partitions with fully-contiguous DMA.

Matmul modes:
  fp32        - native fp32 matmuls (4 cyc/row).
  split3_bf16 - x and w split host-side into bf16 hi + bf16 lo;
                logits = xh*wh + xh*wl + xl*wh accumulated in one PSUM
                bank.  ~fp32-grade routing at bf16 matmul rate.

Self-contained: hardcodes all shapes; only imports the concourse toolchain.
"""
import sys

if "/opt/trn_rl_repo" not in sys.path:
    sys.path.insert(0, "/opt/trn_rl_repo")

import numpy as np

import concourse.bass as bass  # noqa: F401
import concourse.mybir as mybir
import concourse.tile as tile
from concourse import bacc
from concourse.bass_utils import run_bass_kernel_spmd

P = 128            # partitions / tokens per tile
H = 2048           # hidden dim
E = 256            # experts
KO = H // P        # 16 contraction chunks
N_CORES = 8
T_CORE = 4096      # tokens per core
N_TILES = T_CORE // P       # 32 token tiles per core
ST_TOK = 512                # tokens per super-tile
N_ST = T_CORE // ST_TOK     # 8 super-tiles
TPS = ST_TOK // P           # 4 tiles per super-tile

N_GROUP = 8
GSIZE = E // N_GROUP        # 32
TOPK_GROUP = 4
TOP_K = 8
SCALING = 2.5
NEG_BIG = -1.0e30

MATMUL_MODE = "split3_bf16"

f32 = mybir.dt.float32
f16 = mybir.dt.float16
bf16 = mybir.dt.bfloat16
u32 = mybir.dt.uint32
ALU = mybir.AluOpType
ACTF = mybir.ActivationFunctionType
AX = mybir.AxisListType

_CACHED_NC = {}


def build_kernel(mode=MATMUL_MODE):
    nc = bacc.Bacc("TRN2", target_bir_lowering=False, debug=False)

    if mode == "fp32":
        d_x = [nc.dram_tensor("xT", [H, T_CORE], f32, kind="ExternalInput")]
        d_w = [nc.dram_tensor("wT", [H, E], f32, kind="ExternalInput")]
        xdt = f32
    elif mode == "split3_bf16":
        d_x = [nc.dram_tensor("xTh", [H, T_CORE], bf16, kind="ExternalInput"),
               nc.dram_tensor("xTl", [H, T_CORE], bf16, kind="ExternalInput")]
        d_w = [nc.dram_tensor("wTh", [H, E], bf16, kind="ExternalInput"),
               nc.dram_tensor("wTl", [H, E], bf16, kind="ExternalInput")]
        xdt = bf16
    else:
        raise ValueError(mode)
    d_bias = nc.dram_tensor("biasrep", [P, E], f32, kind="ExternalInput")
    d_oidx = nc.dram_tensor("oidx", [P, N_TILES, TOP_K], u32, kind="ExternalOutput")
    d_owgt = nc.dram_tensor("owgt", [P, N_TILES, TOP_K], f32, kind="ExternalOutput")

    with tile.TileContext(nc) as tc:
        with tc.tile_pool(name="const", bufs=1) as cpool, \
             tc.tile_pool(name="xin", bufs=3) as xpool, \
             tc.tile_pool(name="score", bufs=2) as spool, \
             tc.tile_pool(name="small", bufs=2) as mpool, \
             tc.tile_pool(name="psum", bufs=4, space="PSUM") as ppool:

            # ---- constants ----
            if mode == "fp32":
                w_sb = cpool.tile([P, KO, E], f32, name="w0")
                nc.sync.dma_start(w_sb, d_w[0].ap().rearrange("(ko p) e -> p ko e", p=P))
            else:
                whl = cpool.tile([P, KO, 2 * E], bf16, name="whl")
                nc.sync.dma_start(whl[:, :, :E],
                                  d_w[0].ap().rearrange("(ko p) e -> p ko e", p=P))
                nc.sync.dma_start(whl[:, :, E:],
                                  d_w[1].ap().rearrange("(ko p) e -> p ko e", p=P))
            bias_sb = cpool.tile([P, E], f32)
            nc.sync.dma_start(bias_sb, d_bias.ap())
            negbig = cpool.tile([P, 1], f32)
            nc.vector.memset(negbig, NEG_BIG)
            mask_hi = cpool.tile([P, 1], u32)
            nc.vector.memset(mask_hi, 0xFFFFFF00)
            mask_lo = cpool.tile([P, 1], u32)
            nc.vector.memset(mask_lo, 0xFF)
            iota_e = cpool.tile([P, E], u32)
            nc.gpsimd.iota(iota_e, pattern=[[1, E]], base=0, channel_multiplier=0)
            oidx_sb = cpool.tile([P, N_TILES, TOP_K], u32)
            owgt_sb = cpool.tile([P, N_TILES, TOP_K], f32)

            # variable-size super-tiles: small first (fast start), small last (short tail)
            groups = [1, 3, 4, 4, 4, 4, 4, 4, 2, 1, 1]
            assert sum(groups) == N_TILES
            tl0 = 0
            for nt in groups:
                stok = nt * P
                x_sb = []
                for i, d in enumerate(d_x):
                    t = xpool.tile([P, KO, stok], xdt, tag=f"x{i}")
                    src_ap = d.ap().rearrange("(ko p) t -> p ko t", p=P)
                    nh = 2 if nt >= 2 else 1
                    half = stok // nh
                    for hh in range(nh):
                        nc.sync.dma_start(
                            t[:, :, hh * half:(hh + 1) * half],
                            src_ap[:, :, tl0 * P + hh * half:
                                   tl0 * P + (hh + 1) * half])
                    x_sb.append(t)

                # super-tile score tensors [128, nt, 256]
                sg_st = spool.tile([P, nt, E], f32, tag="sg")
                sb_st = spool.tile([P, nt, E], f32, tag="sb")
                sq_st = spool.tile([P, nt, E], f32, tag="sq")
                msf_st = spool.tile([P, nt, E], f32, tag="msf")
                zap_st = spool.tile([P, nt, E], f32, tag="zap")
                ssel_st = spool.tile([P, nt, E], f32, tag="ssel")
                t1g = mpool.tile([P, nt, N_GROUP], f32, tag="t1g")
                t2g = mpool.tile([P, nt, N_GROUP], f32, tag="t2g")
                gs = mpool.tile([P, nt, N_GROUP], f32, tag="gs")
                cc = mpool.tile([P, nt, N_GROUP, N_GROUP], f32, tag="cc")
                c8 = mpool.tile([P, nt, N_GROUP], f32, tag="c8")
                madd = mpool.tile([P, nt, N_GROUP], f32, tag="madd")
                v8 = mpool.tile([P, nt, 8], f32, tag="v8")
                s8 = mpool.tile([P, nt, 8], f32, tag="s8")
                is8 = mpool.tile([P, nt, 8], u32, tag="is8")
                eq = mpool.tile([P, nt, 8, 8], f32, tag="eq")
                sr3 = mpool.tile([P, nt, 8, 8], f32, tag="sr3")
                srank = mpool.tile([P, nt, 8], f32, tag="srank")
                ssum = mpool.tile([P, nt, 1], f32, tag="ssum")
                rs = mpool.tile([P, nt, 1], f32, tag="rs")

                for j in range(nt):
                    tl = tl0 + j
                    tsl = slice(j * P, (j + 1) * P)

                    # ---- logits ----
                    if mode == "fp32":
                        ps = ppool.tile([P, E], f32, tag="ps")
                        for k in range(KO):
                            nc.tensor.matmul(
                                ps, lhsT=x_sb[0][:, k, tsl], rhs=w_sb[:, k, :],
                                start=(k == 0), stop=(k == KO - 1))
                        sig_src = ps
                    else:
                        ps = ppool.tile([P, E], f32, tag="ps")
                        xh, xl = x_sb
                        for k in range(KO):
                            # all three split products accumulate into one bank
                            nc.tensor.matmul(
                                ps, lhsT=xh[:, k, tsl], rhs=whl[:, k, :E],
                                start=(k == 0), stop=False)
                            nc.tensor.matmul(
                                ps, lhsT=xh[:, k, tsl], rhs=whl[:, k, E:],
                                start=False, stop=False)
                            nc.tensor.matmul(
                                ps, lhsT=xl[:, k, tsl], rhs=whl[:, k, :E],
                                start=False, stop=(k == KO - 1))
                        sig_src = ps

                    # ---- sigma = sigmoid(logits) on ACT ----
                    nc.scalar.activation(sg_st[:, j, :], sig_src, ACTF.Sigmoid)

                    # scores_for_choice = sigma + bias            (GPSIMD)
                    nc.gpsimd.tensor_add(sb_st[:, j, :], sg_st[:, j, :], bias_sb)

                # sigma_q: low 8 mantissa bits <- expert id (batched DVE)
                nc.vector.scalar_tensor_tensor(
                    sq_st.bitcast(u32), sg_st.bitcast(u32),
                    mask_hi, iota_e[:, None, :].to_broadcast([P, nt, E]),
                    op0=ALU.bitwise_and, op1=ALU.bitwise_or)

                # ---- group top-2 (batched reduce + per-tile match_replace) ----
                sb4 = sb_st.rearrange("p t (g e) -> p t g e", g=N_GROUP)
                nc.vector.tensor_reduce(out=t1g, in_=sb4, axis=AX.X, op=ALU.max)
                for j in range(nt):
                    nc.vector.match_replace(
                        out=zap_st[:, j, :], in_to_replace=t1g[:, j, :],
                        in_values=sb_st[:, j, :], imm_value=NEG_BIG)
                nc.vector.tensor_reduce(
                    out=t2g, in_=zap_st.rearrange("p t (g e) -> p t g e", g=N_GROUP),
                    axis=AX.X, op=ALU.max)
                nc.vector.tensor_add(gs, t1g, t2g)

                # ---- group rank count + additive mask ----
                nc.vector.tensor_tensor(
                    out=cc,
                    in0=gs[:, :, None, :].to_broadcast([P, nt, N_GROUP, N_GROUP]),
                    in1=gs[:, :, :, None].to_broadcast([P, nt, N_GROUP, N_GROUP]),
                    op=ALU.is_gt)
                nc.vector.tensor_reduce(out=c8, in_=cc, axis=AX.X, op=ALU.add)
                nc.vector.scalar_tensor_tensor(
                    madd, c8, float(TOPK_GROUP) - 0.5,
                    negbig[:, :, None].to_broadcast([P, nt, N_GROUP]),
                    op0=ALU.is_gt, op1=ALU.mult)

                # ---- masked scores ----
                nc.vector.tensor_add(
                    msf_st.rearrange("p t (g e) -> p t g e", g=N_GROUP),
                    sb4,
                    madd[:, :, :, None].to_broadcast([P, nt, N_GROUP, GSIZE]))

                for j in range(nt):
                    tl = tl0 + j
                    # ---- top-8 of masked scores ----
                    nc.vector.max(out=v8[:, j, :], in_=msf_st[:, j, :])
                    nc.vector.max_index(out=oidx_sb[:, tl, :], in_max=v8[:, j, :],
                                        in_values=msf_st[:, j, :])
                    # ---- selected sigma_q: (msf >= v8[7]) * sigma_q ----
                    nc.vector.scalar_tensor_tensor(
                        ssel_st[:, j, :], msf_st[:, j, :], v8[:, j, 7:8],
                        sq_st[:, j, :], op0=ALU.is_ge, op1=ALU.mult)
                    nc.vector.max(out=s8[:, j, :], in_=ssel_st[:, j, :])

                # ---- decode embedded ids, reorder sigmas to score-rank order ----
                nc.vector.tensor_scalar(
                    out=is8, in0=s8.bitcast(u32), scalar1=mask_lo, scalar2=None,
                    op0=ALU.bitwise_and)
                nc.vector.tensor_tensor(
                    out=eq,
                    in0=oidx_sb[:, tl0:tl0 + nt, :, None]
                        .to_broadcast([P, nt, 8, 8]),
                    in1=is8[:, :, None, :].to_broadcast([P, nt, 8, 8]),
                    op=ALU.is_equal)
                nc.vector.tensor_tensor(
                    out=sr3, in0=eq,
                    in1=s8[:, :, None, :].to_broadcast([P, nt, 8, 8]),
                    op=ALU.mult)
                nc.vector.tensor_reduce(out=srank, in_=sr3, axis=AX.X, op=ALU.add)

                # ---- normalize * 2.5 ----
                nc.vector.tensor_reduce(out=ssum, in_=srank, axis=AX.X, op=ALU.add)
                nc.vector.reciprocal(rs, ssum)
                nc.vector.scalar_tensor_tensor(
                    owgt_sb[:, tl0:tl0 + nt, :], srank, SCALING,
                    rs.to_broadcast([P, nt, 8]),
                    op0=ALU.mult, op1=ALU.mult)

                ssl = slice(tl0, tl0 + nt)
                nc.sync.dma_start(d_oidx.ap()[:, ssl, :], oidx_sb[:, ssl, :])
                nc.sync.dma_start(d_owgt.ap()[:, ssl, :], owgt_sb[:, ssl, :])
                tl0 += nt

    nc.compile()
    return nc


def _get_nc(mode):
    if mode not in _CACHED_NC:
        _CACHED_NC[mode] = build_kernel(mode)
    return _CACHED_NC[mode]


def kernel(hidden_states, w, e_score_correction_bias, mode=MATMUL_MODE):
    T = hidden_states.shape[0] * hidden_states.shape[1]
    assert T == N_CORES * T_CORE
    x2 = np.ascontiguousarray(hidden_states.reshape(T, H).astype(np.float32))
    xT = np.ascontiguousarray(x2.T)                       # [H, T]
    wT = np.ascontiguousarray(np.asarray(w, np.float32).T)  # [H, E]
    bias_rep = np.ascontiguousarray(
        np.repeat(np.asarray(e_score_correction_bias, np.float32)[None, :], P, 0))

    if mode == "fp32":
        xs = {"xT": xT}
        ws = {"wT": wT}
    else:
        import ml_dtypes
        bf = ml_dtypes.bfloat16
        xh = xT.astype(bf)
        xl = (xT - xh.astype(np.float32)).astype(bf)
        whh = wT.astype(bf)
        wll = (wT - whh.astype(np.float32)).astype(bf)
        xs = {"xTh": xh, "xTl": xl}
        ws = {"wTh": whh, "wTl": wll}

    nc = _get_nc(mode)
    in_maps = []
    for c in range(N_CORES):
        m = {k: np.ascontiguousarray(v[:, c * T_CORE:(c + 1) * T_CORE])
             for k, v in xs.items()}
        m.update(ws)
        m["biasrep"] = bias_rep
        in_maps.append(m)

    res = run_bass_kernel_spmd(nc, in_maps, core_ids=list(range(N_CORES)))

    idx_parts, wgt_parts = [], []
    for c in range(N_CORES):
        r = res.results[c]
        idx_parts.append(r["oidx"].transpose(1, 0, 2).reshape(T_CORE, TOP_K))
        wgt_parts.append(r["owgt"].transpose(1, 0, 2).reshape(T_CORE, TOP_K))
    topk_idx = np.concatenate(idx_parts, 0).astype(np.int32)
    topk_weight = np.concatenate(wgt_parts, 0).astype(np.float32)
    return topk_idx, topk_weight


# revision 11
# speedup vs baseline: 1.1096x; 1.0693x over previous
"""Trainium2 Bass kernel for DeepSeek-style MoE gate routing.

hidden_states [8, 4096, 2048] f32, w [256, 2048] f32, bias [256] f32
 -> topk_idx [32768, 8] int32, topk_weight [32768, 8] f32

Sharding: tokens split 8 ways across NeuronCores (4096 tokens/core); the
small gate weight + bias are replicated.  x is pre-transposed on the host so
the hidden dim lands on SBUF partitions with fully-contiguous DMA.

Matmul modes:
  fp32        - native fp32 matmuls (4 cyc/row).
  split3_bf16 - x and w split host-side into bf16 hi + bf16 lo;
                logits = xh*wh + xh*wl + xl*wh accumulated in one PSUM
                bank.  ~fp32-grade routing at bf16 matmul rate.

Self-contained: hardcodes all shapes; only imports the concourse toolchain.
"""
import sys

if "/opt/trn_rl_repo" not in sys.path:
    sys.path.insert(0, "/opt/trn_rl_repo")

import numpy as np

import concourse.bass as bass  # noqa: F401
import concourse.mybir as mybir
import concourse.tile as tile
from concourse import bacc
from concourse.bass_utils import run_bass_kernel_spmd

P = 128            # partitions / tokens per tile
H = 2048           # hidden dim
E = 256            # experts
KO = H // P        # 16 contraction chunks
N_CORES = 8
T_CORE = 4096      # tokens per core
N_TILES = T_CORE // P       # 32 token tiles per core
ST_TOK = 512                # tokens per super-tile
N_ST = T_CORE // ST_TOK     # 8 super-tiles
TPS = ST_TOK // P           # 4 tiles per super-tile

N_GROUP = 8
GSIZE = E // N_GROUP        # 32
TOPK_GROUP = 4
TOP_K = 8
SCALING = 2.5
NEG_BIG = -1.0e30

MATMUL_MODE = "split3_bf16"

# token-tile group sizes: small first groups (fast pipeline start) and small
# last groups (short post-matmul tail)
GROUPS = [1, 3, 4, 4, 4, 4, 4, 4, 2, 1, 1]
assert sum(GROUPS) == N_TILES

f32 = mybir.dt.float32
f16 = mybir.dt.float16
bf16 = mybir.dt.bfloat16
u32 = mybir.dt.uint32
ALU = mybir.AluOpType
ACTF = mybir.ActivationFunctionType
AX = mybir.AxisListType

_CACHED_NC = {}


def build_kernel(mode=MATMUL_MODE):
    nc = bacc.Bacc("TRN2", target_bir_lowering=False, debug=False)

    if mode == "fp32":
        d_x = [nc.dram_tensor("xp", [H * T_CORE], f32, kind="ExternalInput")]
        d_w = nc.dram_tensor("wp", [P, KO, E], f32, kind="ExternalInput")
        xdt = f32
        wfree = E
    elif mode == "split3_bf16":
        d_x = [nc.dram_tensor("xph", [H * T_CORE], bf16, kind="ExternalInput"),
               nc.dram_tensor("xpl", [H * T_CORE], bf16, kind="ExternalInput")]
        d_w = nc.dram_tensor("wp", [P, KO, 2 * E], bf16, kind="ExternalInput")
        xdt = bf16
        wfree = 2 * E
    else:
        raise ValueError(mode)
    d_bias = nc.dram_tensor("biasrep", [P, E], f32, kind="ExternalInput")
    d_oidx = nc.dram_tensor("oidx", [P, N_TILES, TOP_K], u32, kind="ExternalOutput")
    d_owgt = nc.dram_tensor("owgt", [P, N_TILES, TOP_K], f32, kind="ExternalOutput")

    with tile.TileContext(nc) as tc:
        with tc.tile_pool(name="const", bufs=1) as cpool, \
             tc.tile_pool(name="xin", bufs=3) as xpool, \
             tc.tile_pool(name="score", bufs=2) as spool, \
             tc.tile_pool(name="small", bufs=2) as mpool, \
             tc.tile_pool(name="psum", bufs=4, space="PSUM") as ppool:

            # ---- constants ----
            whl = cpool.tile([P, KO, wfree], xdt, name="whl")
            nc.sync.dma_start(whl, d_w.ap())
            bias_sb = cpool.tile([P, E], f32)
            nc.sync.dma_start(bias_sb, d_bias.ap())
            negbig = cpool.tile([P, 1], f32)
            nc.vector.memset(negbig, NEG_BIG)
            mask_hi = cpool.tile([P, 1], u32)
            nc.vector.memset(mask_hi, 0xFFFFFF00)
            mask_lo = cpool.tile([P, 1], u32)
            nc.vector.memset(mask_lo, 0xFF)
            iota_e = cpool.tile([P, E], u32)
            nc.gpsimd.iota(iota_e, pattern=[[1, E]], base=0, channel_multiplier=0)
            oidx_sb = cpool.tile([P, N_TILES, TOP_K], u32)
            owgt_sb = cpool.tile([P, N_TILES, TOP_K], f32)

            tl0 = 0
            for nt in GROUPS:
                stok = nt * P
                x_sb = []
                off = tl0 * P * H  # elements before this group in the packed buffer
                for i, d in enumerate(d_x):
                    t = xpool.tile([P, KO, stok], xdt, tag=f"x{i}")
                    nc.sync.dma_start(
                        t, d.ap()[off:off + P * KO * stok]
                            .rearrange("(p ko t) -> p ko t", p=P, ko=KO))
                    x_sb.append(t)
                # super-tile score tensors [128, nt, 256]
                sg_st = spool.tile([P, nt, E], f32, tag="sg")
                sb_st = spool.tile([P, nt, E], f32, tag="sb")
                sq_st = spool.tile([P, nt, E], f32, tag="sq")
                msf_st = spool.tile([P, nt, E], f32, tag="msf")
                zap_st = spool.tile([P, nt, E], f32, tag="zap")
                ssel_st = spool.tile([P, nt, E], f32, tag="ssel")
                t1g = mpool.tile([P, nt, N_GROUP], f32, tag="t1g")
                t2g = mpool.tile([P, nt, N_GROUP], f32, tag="t2g")
                gs = mpool.tile([P, nt, N_GROUP], f32, tag="gs")
                cc = mpool.tile([P, nt, N_GROUP, N_GROUP], f32, tag="cc")
                c8 = mpool.tile([P, nt, N_GROUP], f32, tag="c8")
                madd = mpool.tile([P, nt, N_GROUP], f32, tag="madd")
                v8 = mpool.tile([P, nt, 8], f32, tag="v8")
                s8 = mpool.tile([P, nt, 8], f32, tag="s8")
                is8 = mpool.tile([P, nt, 8], u32, tag="is8")
                eq = mpool.tile([P, nt, 8, 8], f32, tag="eq")
                sr3 = mpool.tile([P, nt, 8, 8], f32, tag="sr3")
                srank = mpool.tile([P, nt, 8], f32, tag="srank")
                ssum = mpool.tile([P, nt, 1], f32, tag="ssum")
                rs = mpool.tile([P, nt, 1], f32, tag="rs")

                for j in range(nt):
                    tl = tl0 + j
                    tsl = slice(j * P, (j + 1) * P)

                    # ---- logits ----
                    if mode == "fp32":
                        ps = ppool.tile([P, E], f32, tag="ps")
                        for k in range(KO):
                            nc.tensor.matmul(
                                ps, lhsT=x_sb[0][:, k, tsl], rhs=whl[:, k, :],
                                start=(k == 0), stop=(k == KO - 1))
                        sig_src = ps
                    else:
                        ps = ppool.tile([P, E], f32, tag="ps")
                        xh, xl = x_sb
                        for k in range(KO):
                            # all three split products accumulate into one bank
                            nc.tensor.matmul(
                                ps, lhsT=xh[:, k, tsl], rhs=whl[:, k, :E],
                                start=(k == 0), stop=False)
                            nc.tensor.matmul(
                                ps, lhsT=xh[:, k, tsl], rhs=whl[:, k, E:],
                                start=False, stop=False)
                            nc.tensor.matmul(
                                ps, lhsT=xl[:, k, tsl], rhs=whl[:, k, :E],
                                start=False, stop=(k == KO - 1))
                        sig_src = ps

                    # ---- sigma = sigmoid(logits) on ACT ----
                    nc.scalar.activation(sg_st[:, j, :], sig_src, ACTF.Sigmoid)

                    # scores_for_choice = sigma + bias            (GPSIMD)
                    nc.gpsimd.tensor_add(sb_st[:, j, :], sg_st[:, j, :], bias_sb)

                # sigma_q: low 8 mantissa bits <- expert id (batched DVE)
                nc.vector.scalar_tensor_tensor(
                    sq_st.bitcast(u32), sg_st.bitcast(u32),
                    mask_hi, iota_e[:, None, :].to_broadcast([P, nt, E]),
                    op0=ALU.bitwise_and, op1=ALU.bitwise_or)

                # ---- group top-2 (batched reduce + per-tile match_replace) ----
                sb4 = sb_st.rearrange("p t (g e) -> p t g e", g=N_GROUP)
                nc.vector.tensor_reduce(out=t1g, in_=sb4, axis=AX.X, op=ALU.max)
                for j in range(nt):
                    nc.vector.match_replace(
                        out=zap_st[:, j, :], in_to_replace=t1g[:, j, :],
                        in_values=sb_st[:, j, :], imm_value=NEG_BIG)
                nc.vector.tensor_reduce(
                    out=t2g, in_=zap_st.rearrange("p t (g e) -> p t g e", g=N_GROUP),
                    axis=AX.X, op=ALU.max)
                nc.vector.tensor_add(gs, t1g, t2g)

                # ---- group rank count + additive mask ----
                nc.vector.tensor_tensor(
                    out=cc,
                    in0=gs[:, :, None, :].to_broadcast([P, nt, N_GROUP, N_GROUP]),
                    in1=gs[:, :, :, None].to_broadcast([P, nt, N_GROUP, N_GROUP]),
                    op=ALU.is_gt)
                nc.vector.tensor_reduce(out=c8, in_=cc, axis=AX.X, op=ALU.add)
                nc.vector.scalar_tensor_tensor(
                    madd, c8, float(TOPK_GROUP) - 0.5,
                    negbig[:, :, None].to_broadcast([P, nt, N_GROUP]),
                    op0=ALU.is_gt, op1=ALU.mult)

                # ---- masked scores ----
                nc.vector.tensor_add(
                    msf_st.rearrange("p t (g e) -> p t g e", g=N_GROUP),
                    sb4,
                    madd[:, :, :, None].to_broadcast([P, nt, N_GROUP, GSIZE]))

                for j in range(nt):
                    tl = tl0 + j
                    # ---- top-8 of masked scores ----
                    nc.vector.max(out=v8[:, j, :], in_=msf_st[:, j, :])
                    nc.vector.max_index(out=oidx_sb[:, tl, :], in_max=v8[:, j, :],
                                        in_values=msf_st[:, j, :])
                    # ---- selected sigma_q: (msf >= v8[7]) * sigma_q ----
                    nc.vector.scalar_tensor_tensor(
                        ssel_st[:, j, :], msf_st[:, j, :], v8[:, j, 7:8],
                        sq_st[:, j, :], op0=ALU.is_ge, op1=ALU.mult)
                    nc.vector.max(out=s8[:, j, :], in_=ssel_st[:, j, :])

                # ---- decode embedded ids, reorder sigmas to score-rank order ----
                nc.vector.tensor_scalar(
                    out=is8, in0=s8.bitcast(u32), scalar1=mask_lo, scalar2=None,
                    op0=ALU.bitwise_and)
                nc.vector.tensor_tensor(
                    out=eq,
                    in0=oidx_sb[:, tl0:tl0 + nt, :, None]
                        .to_broadcast([P, nt, 8, 8]),
                    in1=is8[:, :, None, :].to_broadcast([P, nt, 8, 8]),
                    op=ALU.is_equal)
                nc.vector.tensor_tensor(
                    out=sr3, in0=eq,
                    in1=s8[:, :, None, :].to_broadcast([P, nt, 8, 8]),
                    op=ALU.mult)
                nc.vector.tensor_reduce(out=srank, in_=sr3, axis=AX.X, op=ALU.add)

                # ---- normalize * 2.5 ----
                nc.vector.tensor_reduce(out=ssum, in_=srank, axis=AX.X, op=ALU.add)
                nc.vector.reciprocal(rs, ssum)
                nc.vector.scalar_tensor_tensor(
                    owgt_sb[:, tl0:tl0 + nt, :], srank, SCALING,
                    rs.to_broadcast([P, nt, 8]),
                    op0=ALU.mult, op1=ALU.mult)

                ssl = slice(tl0, tl0 + nt)
                nc.sync.dma_start(d_oidx.ap()[:, ssl, :], oidx_sb[:, ssl, :])
                nc.sync.dma_start(d_owgt.ap()[:, ssl, :], owgt_sb[:, ssl, :])
                tl0 += nt

    nc.compile()
    return nc


def _get_nc(mode):
    if mode not in _CACHED_NC:
        _CACHED_NC[mode] = build_kernel(mode)
    return _CACHED_NC[mode]


def _pack_x(xTc):
    """[H, T_CORE] -> packed 1D so each group's DMA is fully contiguous.

    Block for group (tl0, nt): [P, KO, nt*P] with [p, ko, t] = xTc[ko*P+p, tl0*P+t].
    """
    arr = xTc.reshape(KO, P, T_CORE)
    blocks = []
    tl0 = 0
    for nt in GROUPS:
        blocks.append(np.ascontiguousarray(
            arr[:, :, tl0 * P:(tl0 + nt) * P].transpose(1, 0, 2)).reshape(-1))
        tl0 += nt
    return np.concatenate(blocks)


def _pack_w(wTp):
    """[H, E] -> [P, KO, E] with [p, ko, e] = wTp[ko*P+p, e]."""
    return np.ascontiguousarray(wTp.reshape(KO, P, E).transpose(1, 0, 2))


def kernel(hidden_states, w, e_score_correction_bias, mode=MATMUL_MODE):
    T = hidden_states.shape[0] * hidden_states.shape[1]
    assert T == N_CORES * T_CORE
    x2 = np.ascontiguousarray(hidden_states.reshape(T, H).astype(np.float32))
    xT = np.ascontiguousarray(x2.T)                       # [H, T]
    wT = np.ascontiguousarray(np.asarray(w, np.float32).T)  # [H, E]
    bias_rep = np.ascontiguousarray(
        np.repeat(np.asarray(e_score_correction_bias, np.float32)[None, :], P, 0))

    if mode == "fp32":
        xparts = {"xp": xT}
        wp = _pack_w(wT)
    else:
        import ml_dtypes
        bf = ml_dtypes.bfloat16
        xh = xT.astype(bf)
        xl = (xT - xh.astype(np.float32)).astype(bf)
        xparts = {"xph": xh, "xpl": xl}
        wp = np.concatenate([_pack_w(wT.astype(bf)),
                             _pack_w((wT - wT.astype(bf).astype(np.float32))
                                     .astype(bf))], axis=2)
        wp = np.ascontiguousarray(wp)

    nc = _get_nc(mode)
    in_maps = []
    for c in range(N_CORES):
        m = {k: _pack_x(v[:, c * T_CORE:(c + 1) * T_CORE])
             for k, v in xparts.items()}
        m["wp"] = wp
        m["biasrep"] = bias_rep
        in_maps.append(m)

    res = run_bass_kernel_spmd(nc, in_maps, core_ids=list(range(N_CORES)))

    idx_parts, wgt_parts = [], []
    for c in range(N_CORES):
        r = res.results[c]
        idx_parts.append(r["oidx"].transpose(1, 0, 2).reshape(T_CORE, TOP_K))
        wgt_parts.append(r["owgt"].transpose(1, 0, 2).reshape(T_CORE, TOP_K))
    topk_idx = np.concatenate(idx_parts, 0).astype(np.int32)
    topk_weight = np.concatenate(wgt_parts, 0).astype(np.float32)
    return topk_idx, topk_weight
